# revision 1
# baseline (speedup 1.0000x reference)
"""Trainium2 Bass kernel for AdaptiveLinearWithChannel (moe_routing).

Reference computation:
    w = weight[indices, t]          # (N_sel, D_in, D_out)
    b = bias[indices, t]            # (N_sel, 1, D_out)
    out = x @ w + b                 # (N_sel, PTS, D_out)

Sharding: the selected-channel dim N_sel=256 is split across 8 NeuronCores
(32 channels each, expert-parallel).  The per-channel weight/bias gather is
part of host-side sharding prep; each core then runs 32 independent
(2048x256)@(256x256) GEMMs + bias.

Device layout: the TensorEngine contracts along the partition axis, so x is
staged per-channel as x.T (D_in on partitions).  Each matmul computes an
out.T tile [D_out=128, pts=512] in PSUM (w-slice stationary, x.T moving),
bias is added by VectorE on the way out of PSUM, and the kernel writes out.T
per channel; the host transposes back when unsharding.
"""

import sys

import numpy as np

try:
    import concourse.bacc as bacc
except ImportError:  # fresh dir without the nix sitecustomize on sys.path
    sys.path.insert(0, "/opt/trn_rl_repo")
    import concourse.bacc as bacc

import concourse.mybir as mybir
import concourse.tile as tile
from concourse.bass_utils import run_bass_kernel_spmd

N_SEL = 256
PTS = 2048
D_IN = 256
D_OUT = 256
N_CORES = 8
NCH = N_SEL // N_CORES  # channels per core
P = 128  # partitions

# Compute mode: "f32" (exact), "f32r" (full-rate fp32, reduced mult precision)
COMPUTE = "f32r"
TRACE = False  # test.py flips this to get exec_time_ns

LAST_EXEC_TIME_NS = None

_CACHE = {}


def _build(compute: str):
    f32 = mybir.dt.float32
    mm_dt = mybir.dt.float32r if compute == "f32r" else f32

    nc = bacc.Bacc(None, target_bir_lowering=False)
    xT_ext = nc.declare_dram_parameter("xT", [NCH, D_IN, PTS], f32, isOutput=False)
    w_ext = nc.declare_dram_parameter("w", [NCH, D_IN, D_OUT], f32, isOutput=False)
    bT_ext = nc.declare_dram_parameter("bT", [D_OUT, NCH], f32, isOutput=False)
    out_ext = nc.declare_dram_parameter("outT", [NCH, D_OUT, PTS], f32, isOutput=True)

    KH = D_IN // P  # 2 contraction halves
    MH = D_OUT // P  # 2 output-partition halves
    NPC = PTS // 512  # 4 moving chunks of 512

    with tile.TileContext(nc) as tc:
        with (
            tc.tile_pool(name="xp", bufs=2) as xpool,
            tc.tile_pool(name="wp", bufs=2) as wpool,
            tc.tile_pool(name="bp", bufs=1) as bpool,
            tc.tile_pool(name="op", bufs=8) as opool,
            tc.tile_pool(name="pp", bufs=8, space="PSUM") as pspool,
        ):
            b_sb = bpool.tile([P, MH, NCH], f32, tag="b", name="b_sb")
            for mh in range(MH):
                nc.sync.dma_start(b_sb[:, mh, :], bT_ext[mh * P : (mh + 1) * P, :])

            for ch in range(NCH):
                x_sb = xpool.tile([P, KH, PTS], f32, tag="x", name=f"x{ch}")
                w_sb = wpool.tile([P, KH, D_OUT], f32, tag="w", name=f"w{ch}")
                for kh in range(KH):
                    nc.sync.dma_start(
                        x_sb[:, kh, :], xT_ext[ch, kh * P : (kh + 1) * P, :]
                    )
                    nc.sync.dma_start(
                        w_sb[:, kh, :], w_ext[ch, kh * P : (kh + 1) * P, :]
                    )
                for mh in range(MH):
                    ps = [
                        pspool.tile(
                            [P, 512], f32, tag="ps", name=f"ps{ch}_{mh}_{pc}"
                        )
                        for pc in range(NPC)
                    ]
                    for kh in range(KH):
                        lhsT = w_sb[:, kh, mh * P : (mh + 1) * P].bitcast(mm_dt)
                        for pc in range(NPC):
                            nc.tensor.matmul(
                                ps[pc][:, :],
                                lhsT,
                                x_sb[:, kh, pc * 512 : (pc + 1) * 512].bitcast(mm_dt),
                                start=(kh == 0),
                                stop=(kh == KH - 1),
                            )
                    for pc in range(NPC):
                        o_sb = opool.tile([P, 512], f32, tag="o", name=f"o{ch}_{mh}_{pc}")
                        nc.vector.tensor_scalar_add(
                            o_sb[:, :], ps[pc][:, :], b_sb[:, mh, ch : ch + 1]
                        )
                        nc.sync.dma_start(
                            out_ext[
                                ch, mh * P : (mh + 1) * P, pc * 512 : (pc + 1) * 512
                            ],
                            o_sb[:, :],
                        )

    nc.compile()
    return nc


def _install_ntff_hook():
    """The agent image's antenv lacks axon_hooks; register the NTFF
    profiling hook ourselves so trace=True yields exec_time_ns."""
    try:
        from antenv.axon_hooks import get_axon_ntff_profile_hook  # noqa: F401

        return
    except ImportError:
        pass
    import types

    from trn_agent_boot.trn_boot import _ntff_profile_via_ctypes

    hook = _ntff_profile_via_ctypes("/opt/axon/libaxon_pjrt.so")
    mod = types.ModuleType("antenv.axon_hooks")
    mod.get_axon_ntff_profile_hook = lambda: hook
    mod.set_axon_ntff_profile_hook = lambda h: None
    sys.modules["antenv.axon_hooks"] = mod


def kernel(x, weight, bias, indices, t):
    global LAST_EXEC_TIME_NS

    x = np.asarray(x, dtype=np.float32)
    weight = np.asarray(weight, dtype=np.float32)
    bias = np.asarray(bias, dtype=np.float32)
    idx = np.asarray(indices).astype(np.int64)
    t = int(np.asarray(t))

    # Host-side sharding prep: per-channel gather + transpose.
    w_sel = weight[idx, t]  # (N_sel, D_in, D_out)
    b_sel = bias[idx, t, 0]  # (N_sel, D_out)
    xT = np.ascontiguousarray(x.transpose(0, 2, 1))  # (N_sel, D_in, PTS)

    in_maps = []
    for c in range(N_CORES):
        sl = slice(c * NCH, (c + 1) * NCH)
        in_maps.append(
            {
                "xT": xT[sl],
                "w": np.ascontiguousarray(w_sel[sl]),
                "bT": np.ascontiguousarray(b_sel[sl].T),
            }
        )

    if COMPUTE not in _CACHE:
        _CACHE[COMPUTE] = _build(COMPUTE)
    nc = _CACHE[COMPUTE]

    if TRACE:
        _install_ntff_hook()
    res = run_bass_kernel_spmd(
        nc, in_maps, core_ids=list(range(N_CORES)), trace=TRACE
    )
    LAST_EXEC_TIME_NS = res.exec_time_ns

    outT = np.concatenate(
        [res.results[i]["outT"] for i in range(N_CORES)], axis=0
    )  # (N_sel, D_out, PTS)
    return np.ascontiguousarray(outT.transpose(0, 2, 1))
